# revision 20
# baseline (speedup 1.0000x reference)
"""Trainium2 Bass kernel for BertSelfAttention + LoRA (bs=4, seq=2048, hidden=1024, 16 heads).

Sharding: 8 cores = 4 batches x 2 head-groups. Each core handles one batch and 8
heads (512 of the 1024 hidden output dims). LoRA is folded into the weights on
the host (W_eff = W + scaling * B @ A), and x / W_eff are pre-cast to bf16.

Per-core device kernel (all matmuls bf16, accumulation fp32 in PSUM):
  x16 [2048,1024] -> xT [1024,2048] bf16 in SBUF (host pre-transposed).
  Q^T/K^T projections computed transposed (bias folded via per-partition
  tensor_scalar_add on the PSUM->SBUF cast). V in natural [tok, d'] layout,
  assembled into per-token-tile V' tiles [128, 8*65] with a ones column.
  Attention per head PAIR (head0 on PE rows 0-63, head1 on rows 64-127 ->
  concurrent row groups), fully transposed:
    scoresT[k,q] = K^T(d,k).T @ Q^T(d,q)      (PSUM [128,1024] per head)
    head0: expT = exp(scoresT/8 + mask) on the ACT engine.
    head1: expT via the DVE int16 bit trick (Schraudolph in bf16 domain):
       i16 = trunc(scoresT*23.083 + bdve[k]); bitcast i16 -> bf16 == approx exp.
    The two exps run CONCURRENTLY on different engines -> the per-kt critical
    path is halved vs. two serial ACT activations.
    outT[d',q] += V'[k,d'].T @ expT[k,q]      (PSUM [65,512]; row 64 = denom)
  head0's PV runs inline (lagged 4 kt); head1's exp tiles persist and its PV
  runs as a second pass in the next slot, so only 2 PV PSUM banks are live
  (total 8 banks exactly). Projection work is interleaved into the attention
  kt-loop as filler. PV outputs DMA straight from PSUM to DRAM.
  Host divides by the denominator row and transposes during the gather.
"""

import numpy as np

BS = 4
SEQ = 2048
HID = 1024
HEADS = 16
HD = 64
RANK = 16
LORA_SCALING = 1.0 / RANK

N_CORES = 8
NH = 8          # heads per core
DP = 512        # output dims per core (NH * HD)
P = 128
NT = SEQ // P   # 16 token tiles
NHB = HID // P  # 8 hidden blocks
NG = DP // P    # 4 d' groups (2 heads each)
VW = HD + 1     # 65: V columns + ones column

# DVE fast-exp constants (bf16-domain Schraudolph; DVE f32->i16 truncates)
LOG2E = 1.4426950408889634
DVE_A = LOG2E * 128.0 / 8.0                    # folds the 1/sqrt(64) scale
DVE_C = 366393.0 * 128.0 / 8388608.0           # minimax offset, 2^7 domain
DVE_B0 = 127.0 * 128.0 - DVE_C + 0.5           # +0.5: truncation -> rounding
DVE_BM = LOG2E * 128.0                         # mask multiplier

_CACHE = {}


def _build():
    import concourse.bass as bass
    import concourse.tile as tile
    from concourse import bacc, mybir

    f32 = mybir.dt.float32
    bf16 = mybir.dt.bfloat16
    i16 = mybir.dt.int16
    Exp = mybir.ActivationFunctionType.Exp
    Alu = mybir.AluOpType

    nc = bacc.Bacc("TRN2", target_bir_lowering=False, debug=False,
                   num_devices=N_CORES)

    xTin = nc.dram_tensor("xT16", [HID, SEQ], bf16, kind="ExternalInput").ap()
    wTin = [nc.dram_tensor(f"wT{n}", [HID, DP], bf16, kind="ExternalInput").ap()
            for n in "qkv"]
    bias_qk = [nc.dram_tensor(f"b{n}", [P, NG], f32, kind="ExternalInput").ap()
               for n in "qk"]
    mask = nc.dram_tensor("mask", [P, NT], f32, kind="ExternalInput").ap()
    bdve = nc.dram_tensor("bdve", [P, NT], f32, kind="ExternalInput").ap()
    out = nc.dram_tensor("out", [NH, VW, SEQ], f32, kind="ExternalOutput").ap()

    with tile.TileContext(nc) as tc:
        with (
            tc.tile_pool(name="consts", bufs=1) as cpool,
            tc.tile_pool(name="xT", bufs=1) as xT_pool,
            tc.tile_pool(name="wT", bufs=1) as wT_pool,
            tc.tile_pool(name="qkT", bufs=1) as qkT_pool,
            tc.tile_pool(name="vp", bufs=1) as vp_pool,
            tc.tile_pool(name="expp", bufs=8) as exp_pool,
            tc.tile_pool(name="exp1", bufs=18) as exp1_pool,
            tc.tile_pool(name="outp", bufs=3) as out_pool,
            tc.tile_pool(name="ps_proj", bufs=2, space="PSUM") as ps_proj,
            tc.tile_pool(name="ps_sc", bufs=4, space="PSUM") as ps_sc,
            tc.tile_pool(name="ps_pv", bufs=2, space="PSUM") as ps_pv,
        ):
            # ---- constants (off the sync queue so wk/x0 issue first) ----
            mask_t = cpool.tile([P, NT], f32, tag="mask", name="mask_t")
            nc.scalar.dma_start(mask_t[:], mask[:, :])
            bdve_t = cpool.tile([P, NT], f32, tag="bdve", name="bdve_t")
            nc.scalar.dma_start(bdve_t[:], bdve[:, :])
            bias_t = []
            for i in range(2):
                bt = cpool.tile([P, NG], f32, tag=f"bias{i}", name=f"bias{i}")
                nc.scalar.dma_start(bt[:], bias_qk[i][:, :])
                bias_t.append(bt)

            # ---- plain contiguous loads (x and W arrive pre-transposed) ----
            wT = [[wT_pool.tile([P, DP], bf16, tag=f"wT{w}_{c}",
                                name=f"wT{w}_{c}")
                   for c in range(NHB)] for w in range(3)]
            xT = [xT_pool.tile([P, SEQ], bf16, tag=f"xT{c}", name=f"xT{c}")
                  for c in range(NHB)]

            def load_w(w, c, eng):
                eng.dma_start(wT[w][c][:], wTin[w][c * P:(c + 1) * P, :])

            # PE warm-up burst: ~4us of tiny matmuls during the initial
            # DMA window ramps the PE p-state before real work lands
            warm = cpool.tile([64, 128], bf16, tag="warm", name="warm")
            nc.gpsimd.memset(warm[:], 0.0078125)
            wps = ps_proj.tile([64, 128], f32, tag="proj", name="wps")
            for i in range(36):
                nc.tensor.matmul(wps[:], warm[:, 0:64], warm[:],
                                 start=(i == 0), stop=(i == 35))
            wsink = cpool.tile([64, 128], f32, tag="wsink", name="wsink")
            nc.vector.tensor_copy(wsink[:], wps[:])

            # K-weights and x chunk 0 first (first QK unit's operands),
            # issued from the sync queue; the bulk (wq, wv, x chunks 1-3)
            # goes through the otherwise-idle gpsimd queue so SP's per-DMA
            # issue cost doesn't serialize the startup.
            for c in range(NHB):
                load_w(1, c, nc.sync)
                nc.sync.dma_start(xT[c][:, 0:512], xTin[c * P:(c + 1) * P, 0:512])
            for c in range(NHB):
                load_w(2, c, nc.gpsimd)
            for c in range(NHB):
                load_w(0, c, nc.sync)
            for c in range(NHB):
                nc.gpsimd.dma_start(xT[c][:, 512:1024],
                                    xTin[c * P:(c + 1) * P, 512:1024])
            for c in range(NHB):
                nc.gpsimd.dma_start(xT[c][:, 1024:SEQ],
                                    xTin[c * P:(c + 1) * P, 1024:SEQ])

            qkT = [[qkT_pool.tile([P, SEQ], bf16, tag=f"qkT{w}_{g}",
                                  name=f"qkT{w}_{g}")
                    for g in range(NG)] for w in range(2)]
            vp = [vp_pool.tile([P, NH * VW], bf16, tag=f"vp{tt}",
                               name=f"vp{tt}") for tt in range(NT)]

            def emit_v_unit(tt):
                """V projection for token tile tt + V' assembly (ones col)."""
                ps = ps_proj.tile([P, DP], f32, tag="proj", name="ps_v")
                for p in range(NHB):
                    nc.tensor.matmul(ps[:],
                                     xT[p][:, tt * P:(tt + 1) * P],
                                     wT[2][p][:],
                                     start=(p == 0), stop=(p == NHB - 1))
                nc.gpsimd.memset(vp[tt][:], 1.0)
                dst = vp[tt][:].rearrange("p (h c) -> p h c", c=VW)[:, :, 0:HD]
                nc.scalar.activation(dst,
                                     ps[:].rearrange("p (h c) -> p h c", c=HD),
                                     mybir.ActivationFunctionType.Copy)

            def qk_unit_halves(g, w, tc):
                """Q^T/K^T projection for group g, proj w, tokchunk tc,
                split into two ~4-matmul filler closures."""
                gs = slice(g * P, (g + 1) * P)
                cell = {}

                def first():
                    cell["ps"] = ps_proj.tile([P, 512], f32, tag="proj",
                                              name="ps_qk")
                    for p in range(4):
                        nc.tensor.matmul(cell["ps"][:], wT[w][p][:, gs],
                                         xT[p][:, tc * 512:(tc + 1) * 512],
                                         start=(p == 0), stop=False)

                def second():
                    for p in range(4, NHB):
                        nc.tensor.matmul(cell["ps"][:], wT[w][p][:, gs],
                                         xT[p][:, tc * 512:(tc + 1) * 512],
                                         start=False, stop=(p == NHB - 1))
                    nc.vector.tensor_scalar_add(
                        qkT[w][g][:, tc * 512:(tc + 1) * 512],
                        cell["ps"][:],
                        bias_t[w][:, g:g + 1])

                return [first, second]

            def emit_qk_unit(g, w, tc):
                for f in qk_unit_halves(g, w, tc):
                    f()

            # upfront: just enough projection for the first scores + exps
            emit_qk_unit(0, 1, 0)   # K^T group0 tok 0:512 (covers kt 0..3)
            emit_qk_unit(0, 0, 0)   # Q^T group0 tok 0:512
            emit_qk_unit(0, 0, 1)   # Q^T group0 tok 512:1024

            # per-(g,qb) filler schedules: step -> [closures]
            def sched_of(g, qb):
                s = {}

                def put(step, *cl):
                    s.setdefault(step, []).extend(cl)

                if (g, qb) == (0, 0):
                    for tt in range(4):
                        put(tt, lambda tt=tt: emit_v_unit(tt))
                    for tt in range(4, NT):
                        put(tt - 4, lambda tt=tt: emit_v_unit(tt))
                    put(0, *qk_unit_halves(0, 1, 1))    # K tc1 (kt 4..7)
                    put(2, *qk_unit_halves(0, 1, 2))    # K tc2 (kt 8..11)
                    put(4, *qk_unit_halves(0, 1, 3))    # K tc3
                    put(6, *qk_unit_halves(0, 0, 2))    # Q tc2 (qb1)
                    put(8, *qk_unit_halves(0, 0, 3))    # Q tc3
                elif (g, qb) == (0, 1):
                    halves = []
                    for w in (0, 1):
                        for tc in range(4):
                            halves += qk_unit_halves(1, w, tc)
                    for i, f in enumerate(halves):
                        put(i, f)
                elif qb == 0 and 0 < g < NG - 1:
                    # QK(g+1) spread over (g, qb0) steps 8.. and (g, qb1)
                    halves = []
                    for w in (0, 1):
                        for tc in range(4):
                            halves += qk_unit_halves(g + 1, w, tc)
                    for i, f in enumerate(halves[:8]):
                        put(8 + i, f)
                    s["rest"] = halves[8:]
                return s

            pending = None          # (h1, qb, et1 tiles) awaiting PV pass
            carry = []              # leftover filler closures from qb0

            def emit_pvh1_chunk(pend, j, pvt1):
                h1p, qbp, et1p = pend
                vb1 = h1p * VW
                for kt in range(4 * j, 4 * j + 4):
                    for qc in range(2):
                        nc.tensor.matmul(pvt1[qc][:],
                                         vp[kt][:, vb1:vb1 + VW],
                                         et1p[kt][:, qc * 512:(qc + 1) * 512],
                                         start=(kt == 0),
                                         stop=(kt == NT - 1))

            def emit_out(h, qb_, qc, pvt):
                """PSUM [65,512] -> SBUF (on ACT; it has slack) -> DRAM."""
                ot = out_pool.tile([VW, 512], f32, tag="ot", name="ot")
                nc.scalar.activation(ot[:], pvt[:],
                                     mybir.ActivationFunctionType.Copy)
                q0 = qb_ * 1024 + qc * 512
                nc.sync.dma_start(out[h][:, q0:q0 + 512], ot[:])

            def emit_pvh1_outs(pend, pvt1):
                h1p, qbp, _ = pend
                for qc in range(2):
                    emit_out(h1p, qbp, qc, pvt1[qc])

            def emit_exp(dst, sc, kt, use_dve):
                """exp of one [128,512] scores PSUM tile on one engine."""
                if use_dve:
                    nc.vector.tensor_scalar(dst.bitcast(i16), sc[:],
                                            DVE_A, bdve_t[:, kt:kt + 1],
                                            Alu.mult, Alu.add)
                else:
                    nc.scalar.activation(dst, sc[:], Exp,
                                         bias=mask_t[:, kt:kt + 1],
                                         scale=0.125)

            for g in range(NG):
                h0, h1 = 2 * g, 2 * g + 1
                sl0, sl1 = slice(0, HD), slice(HD, P)
                for qb in range(2):
                    sched = sched_of(g, qb)
                    if qb == 1 and carry:
                        # leftover QK(g+1) halves into qb1 steps 4..
                        for i, f in enumerate(carry):
                            sched.setdefault(4 + i, []).append(f)
                        carry = []
                    if "rest" in sched:
                        carry = sched.pop("rest")
                    et0s = []
                    et1 = []
                    pvt0 = None
                    pvt1_prev = None
                    inline_h1 = (g == NG - 1 and qb == 1)
                    pvt1_inl = None
                    for kt in range(NT):
                        if kt == 0 and inline_h1:
                            pvt1_inl = [ps_proj.tile([VW, 512], f32,
                                                     tag="proj", name="pvi")
                                        for _ in range(2)]
                        if kt == 0 and pending is not None:
                            pvt1_prev = [ps_pv.tile([VW, 512], f32, tag="pv",
                                                    name="pv1")
                                         for _ in range(2)]
                        if kt < 4 and pending is not None:
                            emit_pvh1_chunk(pending, kt, pvt1_prev)
                            if kt == 3:
                                emit_pvh1_outs(pending, pvt1_prev)
                                pending = None
                        for f in sched.get(kt, ()):
                            f()
                        ks = slice(kt * P, (kt + 1) * P)
                        # scores in four 1-bank [128,512] PSUM tiles so each
                        # gets ONE exp engine and frees independently (the
                        # kt+1 scores WAR only the same slot of kt).
                        scs = [ps_sc.tile([P, 512], f32, tag="sc",
                                          name=f"sc{i}")
                               for i in range(4)]
                        for qh in range(2):
                            q0 = qb * 1024 + qh * 512
                            qs = slice(q0, q0 + 512)
                            nc.tensor.matmul(scs[qh][:], qkT[1][g][sl0, ks],
                                             qkT[0][g][sl0, qs],
                                             start=True, stop=True)
                            nc.tensor.matmul(scs[2 + qh][:],
                                             qkT[1][g][sl1, ks],
                                             qkT[0][g][sl1, qs],
                                             start=True, stop=True)
                        # per-tile engine alternates over (kt, head, qh) so
                        # both engines stay ~50% and every (head, qh) region
                        # mixes both exp flavors across kt.
                        et0 = exp_pool.tile([P, 1024], bf16, tag="exp",
                                            name="et0")
                        et1k = exp1_pool.tile([P, 1024], bf16, tag="exp1",
                                              name="et1")
                        for h, ett in ((0, et0), (1, et1k)):
                            for qh in range(2):
                                idx = (kt * 4 + h * 2 + qh) * 7 % 16
                                emit_exp(ett[:, qh * 512:(qh + 1) * 512],
                                         scs[2 * h + qh], kt,
                                         use_dve=(idx < 7))
                        et0s.append(et0)
                        et1.append(et1k)
                        if kt == 4:
                            pvt0 = [ps_pv.tile([VW, 512], f32, tag="pv",
                                               name="pv0") for _ in range(2)]
                        if kt >= 4:
                            ktl = kt - 4   # lagged PV for head0
                            vb0 = h0 * VW
                            for qc in range(2):
                                nc.tensor.matmul(
                                    pvt0[qc][:],
                                    vp[ktl][:, vb0:vb0 + VW],
                                    et0s[ktl][:, qc * 512:(qc + 1) * 512],
                                    start=(ktl == 0), stop=False)
                        if inline_h1:
                            vb1 = h1 * VW
                            for qc in range(2):
                                nc.tensor.matmul(
                                    pvt1_inl[qc][:],
                                    vp[kt][:, vb1:vb1 + VW],
                                    et1[kt][:, qc * 512:(qc + 1) * 512],
                                    start=(kt == 0), stop=(kt == NT - 1))
                    for kt in range(NT - 4, NT):
                        vb0 = h0 * VW
                        for qc in range(2):
                            nc.tensor.matmul(pvt0[qc][:],
                                             vp[kt][:, vb0:vb0 + VW],
                                             et0s[kt][:, qc * 512:(qc + 1) * 512],
                                             start=False, stop=(kt == NT - 1))
                    for qc in range(2):
                        emit_out(h0, qb, qc, pvt0[qc])
                    if inline_h1:
                        for qc in range(2):
                            emit_out(h1, qb, qc, pvt1_inl[qc])
                    else:
                        pending = (h1, qb, et1)

    nc.compile()
    return nc


def _get_nc():
    if "nc" not in _CACHE:
        _CACHE["nc"] = _build()
    return _CACHE["nc"]


def kernel(hidden_states, attention_mask, Wq, bq, Aq, Bq, Wk, bk, Ak, Bk,
           Wv, bv, Av, Bv):
    from concourse import bass_utils
    import ml_dtypes
    import os

    nc = _get_nc()
    bf = ml_dtypes.bfloat16

    hs = np.asarray(hidden_states, dtype=np.float32)
    am = np.asarray(attention_mask, dtype=np.float32)
    weff = {}
    for n, W, A, B in (("q", Wq, Aq, Bq), ("k", Wk, Ak, Bk), ("v", Wv, Av, Bv)):
        W = np.asarray(W, dtype=np.float32)
        A = np.asarray(A, dtype=np.float32)
        B = np.asarray(B, dtype=np.float32)
        weff[n] = (W + LORA_SCALING * (B @ A)).astype(bf)
    biases = {"q": np.asarray(bq, np.float32), "k": np.asarray(bk, np.float32),
              "v": np.asarray(bv, np.float32)}
    hs16T = [np.ascontiguousarray(hs[b].T.astype(bf)) for b in range(BS)]

    in_maps = []
    for c in range(N_CORES):
        b, hg = divmod(c, 2)
        rows = slice(hg * DP, (hg + 1) * DP)
        mask_cols = np.ascontiguousarray(am[b, 0, 0].reshape(NT, P).T)
        m = {
            "xT16": hs16T[b],
            "mask": mask_cols,
            "bdve": np.ascontiguousarray(DVE_B0 + DVE_BM * mask_cols),
        }
        for n in ("q", "k", "v"):
            m[f"wT{n}"] = np.ascontiguousarray(weff[n][rows].T)
        for n in ("q", "k"):
            m[f"b{n}"] = np.ascontiguousarray(
                biases[n][rows].reshape(NG, P).T)
        in_maps.append(m)

    trace = bool(int(os.environ.get("BASS_KERNEL_TRACE", "0")))
    res = bass_utils.run_bass_kernel_spmd(nc, in_maps,
                                          core_ids=list(range(N_CORES)),
                                          trace=trace)
    _CACHE["last_results"] = res

    output = np.empty((BS, SEQ, HID), dtype=np.float32)
    for c in range(N_CORES):
        b, hg = divmod(c, 2)
        r = res.results[c]["out"]                      # [NH, 65, SEQ]
        o = r[:, :HD, :] / r[:, HD:HD + 1, :]          # [NH, 64, SEQ]
        rows = slice(hg * DP, (hg + 1) * DP)
        output[b, :, rows] = (o.transpose(2, 0, 1).reshape(SEQ, DP)
                              + biases["v"][rows][None, :])
    return output


# revision 26
# speedup vs baseline: 1.0121x; 1.0121x over previous
"""Trainium2 Bass kernel for BertSelfAttention + LoRA (bs=4, seq=2048, hidden=1024, 16 heads).

Sharding: 8 cores = 4 batches x 2 head-groups. Each core handles one batch and 8
heads (512 of the 1024 hidden output dims). LoRA is folded into the weights on
the host (W_eff = W + scaling * B @ A), and x / W_eff are pre-cast to bf16.

Per-core device kernel (all matmuls bf16, accumulation fp32 in PSUM):
  x16 [2048,1024] -> xT [1024,2048] bf16 in SBUF (host pre-transposed).
  Q^T/K^T projections computed transposed (bias folded via per-partition
  tensor_scalar_add on the PSUM->SBUF cast). V in natural [tok, d'] layout,
  assembled into per-token-tile V' tiles [128, 8*65] with a ones column.
  Attention per head PAIR (head0 on PE rows 0-63, head1 on rows 64-127 ->
  concurrent row groups), fully transposed:
    scoresT[k,q] = K^T(d,k).T @ Q^T(d,q)      (PSUM [128,1024] per head)
    head0: expT = exp(scoresT/8 + mask) on the ACT engine.
    head1: expT via the DVE int16 bit trick (Schraudolph in bf16 domain):
       i16 = trunc(scoresT*23.083 + bdve[k]); bitcast i16 -> bf16 == approx exp.
    The two exps run CONCURRENTLY on different engines -> the per-kt critical
    path is halved vs. two serial ACT activations.
    outT[d',q] += V'[k,d'].T @ expT[k,q]      (PSUM [65,512]; row 64 = denom)
  head0's PV runs inline (lagged 4 kt); head1's exp tiles persist and its PV
  runs as a second pass in the next slot, so only 2 PV PSUM banks are live
  (total 8 banks exactly). Projection work is interleaved into the attention
  kt-loop as filler. PV outputs DMA straight from PSUM to DRAM.
  Host divides by the denominator row and transposes during the gather.
"""

import numpy as np

BS = 4
SEQ = 2048
HID = 1024
HEADS = 16
HD = 64
RANK = 16
LORA_SCALING = 1.0 / RANK

N_CORES = 8
NH = 8          # heads per core
DP = 512        # output dims per core (NH * HD)
P = 128
NT = SEQ // P   # 16 token tiles
NHB = HID // P  # 8 hidden blocks
NG = DP // P    # 4 d' groups (2 heads each)
VW = HD + 1     # 65: V columns + ones column

# DVE fast-exp constants (bf16-domain Schraudolph; DVE f32->i16 truncates)
LOG2E = 1.4426950408889634
DVE_A = LOG2E * 128.0 / 8.0                    # folds the 1/sqrt(64) scale
DVE_C = 366393.0 * 128.0 / 8388608.0           # minimax offset, 2^7 domain
DVE_B0 = 127.0 * 128.0 - DVE_C + 0.5           # +0.5: truncation -> rounding
DVE_BM = LOG2E * 128.0                         # mask multiplier

_CACHE = {}


def _build():
    import concourse.bass as bass
    import concourse.tile as tile
    from concourse import bacc, mybir

    f32 = mybir.dt.float32
    bf16 = mybir.dt.bfloat16
    i16 = mybir.dt.int16
    Exp = mybir.ActivationFunctionType.Exp
    Alu = mybir.AluOpType

    nc = bacc.Bacc("TRN2", target_bir_lowering=False, debug=False,
                   num_devices=N_CORES)

    xTin = nc.dram_tensor("xT16", [HID, SEQ], bf16, kind="ExternalInput").ap()
    wTin = [nc.dram_tensor(f"wT{n}", [HID, DP], bf16, kind="ExternalInput").ap()
            for n in "qkv"]
    bias_qk = [nc.dram_tensor(f"b{n}", [P, NG], f32, kind="ExternalInput").ap()
               for n in "qk"]
    mask = nc.dram_tensor("mask", [P, NT], f32, kind="ExternalInput").ap()
    bdve = nc.dram_tensor("bdve", [P, NT], f32, kind="ExternalInput").ap()
    out = nc.dram_tensor("out", [NH, VW, SEQ], f32, kind="ExternalOutput").ap()

    with tile.TileContext(nc) as tc:
        with (
            tc.tile_pool(name="consts", bufs=1) as cpool,
            tc.tile_pool(name="xT", bufs=1) as xT_pool,
            tc.tile_pool(name="wT", bufs=1) as wT_pool,
            tc.tile_pool(name="qkT", bufs=1) as qkT_pool,
            tc.tile_pool(name="vp", bufs=1) as vp_pool,
            tc.tile_pool(name="expp", bufs=8) as exp_pool,
            tc.tile_pool(name="exp1", bufs=18) as exp1_pool,
            tc.tile_pool(name="outp", bufs=3) as out_pool,
            tc.tile_pool(name="ps_proj", bufs=1, space="PSUM") as ps_proj,
            tc.tile_pool(name="ps_sc", bufs=5, space="PSUM") as ps_sc,
            tc.tile_pool(name="ps_pv", bufs=2, space="PSUM") as ps_pv,
        ):
            # ---- constants (off the sync queue so wk/x0 issue first) ----
            mask_t = cpool.tile([P, NT], f32, tag="mask", name="mask_t")
            nc.scalar.dma_start(mask_t[:], mask[:, :])
            bdve_t = cpool.tile([P, NT], f32, tag="bdve", name="bdve_t")
            nc.scalar.dma_start(bdve_t[:], bdve[:, :])
            bias_t = []
            for i in range(2):
                bt = cpool.tile([P, NG], f32, tag=f"bias{i}", name=f"bias{i}")
                nc.scalar.dma_start(bt[:], bias_qk[i][:, :])
                bias_t.append(bt)

            # ---- plain contiguous loads (x and W arrive pre-transposed) ----
            wT = [[wT_pool.tile([P, DP], bf16, tag=f"wT{w}_{c}",
                                name=f"wT{w}_{c}")
                   for c in range(NHB)] for w in range(3)]
            xT = [xT_pool.tile([P, SEQ], bf16, tag=f"xT{c}", name=f"xT{c}")
                  for c in range(NHB)]

            def load_w(w, c, eng):
                eng.dma_start(wT[w][c][:], wTin[w][c * P:(c + 1) * P, :])

            # PE warm-up burst: ~4us of tiny matmuls during the initial
            # DMA window ramps the PE p-state before real work lands
            warm = cpool.tile([64, 128], bf16, tag="warm", name="warm")
            nc.gpsimd.memset(warm[:], 0.0078125)
            wps = ps_proj.tile([64, 128], f32, tag="proj", name="wps")
            for i in range(36):
                nc.tensor.matmul(wps[:], warm[:, 0:64], warm[:],
                                 start=(i == 0), stop=(i == 35))
            wsink = cpool.tile([64, 128], f32, tag="wsink", name="wsink")
            nc.vector.tensor_copy(wsink[:], wps[:])

            # K-weights and x chunk 0 first (first QK unit's operands),
            # issued from the sync queue; the bulk (wq, wv, x chunks 1-3)
            # goes through the otherwise-idle gpsimd queue so SP's per-DMA
            # issue cost doesn't serialize the startup.
            for c in range(NHB):
                load_w(1, c, nc.sync)
                nc.sync.dma_start(xT[c][:, 0:512], xTin[c * P:(c + 1) * P, 0:512])
            for c in range(NHB):
                load_w(2, c, nc.gpsimd)
            for c in range(NHB):
                load_w(0, c, nc.sync)
            for c in range(NHB):
                nc.gpsimd.dma_start(xT[c][:, 512:1024],
                                    xTin[c * P:(c + 1) * P, 512:1024])
            for c in range(NHB):
                nc.gpsimd.dma_start(xT[c][:, 1024:SEQ],
                                    xTin[c * P:(c + 1) * P, 1024:SEQ])

            qkT = [[qkT_pool.tile([P, SEQ], bf16, tag=f"qkT{w}_{g}",
                                  name=f"qkT{w}_{g}")
                    for g in range(NG)] for w in range(2)]
            vp = [vp_pool.tile([P, NH * VW], bf16, tag=f"vp{tt}",
                               name=f"vp{tt}") for tt in range(NT)]

            def emit_v_unit(tt):
                """V projection for token tile tt + V' assembly (ones col).
                Allocates from the 5-deep sc pool so V units don't serialize
                on the single QK-proj bank."""
                ps = ps_sc.tile([P, DP], f32, tag="sc", name="ps_v")
                for p in range(NHB):
                    nc.tensor.matmul(ps[:],
                                     xT[p][:, tt * P:(tt + 1) * P],
                                     wT[2][p][:],
                                     start=(p == 0), stop=(p == NHB - 1))
                nc.gpsimd.memset(vp[tt][:], 1.0)
                dst = vp[tt][:].rearrange("p (h c) -> p h c", c=VW)[:, :, 0:HD]
                nc.scalar.activation(dst,
                                     ps[:].rearrange("p (h c) -> p h c", c=HD),
                                     mybir.ActivationFunctionType.Copy)

            def qk_unit_halves(g, w, tc):
                """Q^T/K^T projection for group g, proj w, tokchunk tc,
                split into two ~4-matmul filler closures."""
                gs = slice(g * P, (g + 1) * P)
                cell = {}

                def first():
                    cell["ps"] = ps_proj.tile([P, 512], f32, tag="proj",
                                              name="ps_qk")
                    for p in range(4):
                        nc.tensor.matmul(cell["ps"][:], wT[w][p][:, gs],
                                         xT[p][:, tc * 512:(tc + 1) * 512],
                                         start=(p == 0), stop=False)

                def second():
                    for p in range(4, NHB):
                        nc.tensor.matmul(cell["ps"][:], wT[w][p][:, gs],
                                         xT[p][:, tc * 512:(tc + 1) * 512],
                                         start=False, stop=(p == NHB - 1))
                    nc.vector.tensor_scalar_add(
                        qkT[w][g][:, tc * 512:(tc + 1) * 512],
                        cell["ps"][:],
                        bias_t[w][:, g:g + 1])

                return [first, second]

            def emit_qk_unit(g, w, tc):
                for f in qk_unit_halves(g, w, tc):
                    f()

            # upfront: just enough projection for the first scores + exps
            emit_qk_unit(0, 1, 0)   # K^T group0 tok 0:512 (covers kt 0..3)
            emit_qk_unit(0, 0, 0)   # Q^T group0 tok 0:512
            emit_qk_unit(0, 0, 1)   # Q^T group0 tok 512:1024

            # per-(g,qb) filler schedules: step -> [closures]
            def sched_of(g, qb):
                s = {}

                def put(step, *cl):
                    s.setdefault(step, []).extend(cl)

                if (g, qb) == (0, 0):
                    for tt in range(4):
                        put(tt, lambda tt=tt: emit_v_unit(tt))
                    for tt in range(4, NT):
                        put(tt - 4, lambda tt=tt: emit_v_unit(tt))
                    put(0, *qk_unit_halves(0, 1, 1))    # K tc1 (kt 4..7)
                    put(2, *qk_unit_halves(0, 1, 2))    # K tc2 (kt 8..11)
                    put(4, *qk_unit_halves(0, 1, 3))    # K tc3
                    put(6, *qk_unit_halves(0, 0, 2))    # Q tc2 (qb1)
                    put(8, *qk_unit_halves(0, 0, 3))    # Q tc3
                elif (g, qb) == (0, 1):
                    halves = []
                    for w in (0, 1):
                        for tc in range(4):
                            halves += qk_unit_halves(1, w, tc)
                    for i, f in enumerate(halves):
                        put(i, f)
                elif qb == 0 and 0 < g < NG - 1:
                    # QK(g+1) spread over (g, qb0) steps 8.. and (g, qb1)
                    halves = []
                    for w in (0, 1):
                        for tc in range(4):
                            halves += qk_unit_halves(g + 1, w, tc)
                    for i, f in enumerate(halves[:8]):
                        put(8 + i, f)
                    s["rest"] = halves[8:]
                return s

            pending = None          # (h1, qb, et1 tiles) awaiting PV pass
            carry = []              # leftover filler closures from qb0

            def emit_pvh1_chunk(pend, j, pvt1):
                h1p, qbp, et1p = pend
                vb1 = h1p * VW
                for kt in range(4 * j, 4 * j + 4):
                    for qc in range(2):
                        nc.tensor.matmul(pvt1[qc][:],
                                         vp[kt][:, vb1:vb1 + VW],
                                         et1p[kt][:, qc * 512:(qc + 1) * 512],
                                         start=(kt == 0),
                                         stop=(kt == NT - 1))

            def emit_out(h, qb_, qc, pvt, on_dve=False):
                """PSUM [65,512] -> SBUF (ACT; DVE when asked) -> DRAM."""
                ot = out_pool.tile([VW, 512], f32, tag="ot", name="ot")
                if on_dve:
                    nc.vector.tensor_copy(ot[:], pvt[:])
                else:
                    nc.scalar.activation(ot[:], pvt[:],
                                         mybir.ActivationFunctionType.Copy)
                q0 = qb_ * 1024 + qc * 512
                nc.sync.dma_start(out[h][:, q0:q0 + 512], ot[:])

            def emit_pvh1_outs(pend, pvt1):
                h1p, qbp, _ = pend
                for qc in range(2):
                    emit_out(h1p, qbp, qc, pvt1[qc])

            def emit_exp(dst, sc, kt, use_dve):
                """exp of one [128,512] scores PSUM tile on one engine."""
                if use_dve:
                    nc.vector.tensor_scalar(dst.bitcast(i16), sc[:],
                                            DVE_A, bdve_t[:, kt:kt + 1],
                                            Alu.mult, Alu.add)
                else:
                    nc.scalar.activation(dst, sc[:], Exp,
                                         bias=mask_t[:, kt:kt + 1],
                                         scale=0.125)

            for g in range(NG):
                h0, h1 = 2 * g, 2 * g + 1
                sl0, sl1 = slice(0, HD), slice(HD, P)
                for qb in range(2):
                    sched = sched_of(g, qb)
                    if qb == 1 and carry:
                        # leftover QK(g+1) halves into qb1 steps 4..
                        for i, f in enumerate(carry):
                            sched.setdefault(4 + i, []).append(f)
                        carry = []
                    if "rest" in sched:
                        carry = sched.pop("rest")
                    et0s = []
                    et1 = []
                    pvt0 = None
                    pvt1_prev = None
                    inline_h1 = (g == NG - 1 and qb == 1)
                    pvt1_inl = None
                    for kt in range(NT):
                        if kt == 0 and inline_h1:
                            pvt1_inl = [ps_sc.tile([VW, 512], f32,
                                                   tag="sc", name="pvi")
                                        for _ in range(2)]
                        if kt == 0 and pending is not None:
                            pvt1_prev = [ps_pv.tile([VW, 512], f32, tag="pv",
                                                    name="pv1")
                                         for _ in range(2)]
                        if kt < 4 and pending is not None:
                            emit_pvh1_chunk(pending, kt, pvt1_prev)
                            if kt == 3:
                                emit_pvh1_outs(pending, pvt1_prev)
                                pending = None
                        for f in sched.get(kt, ()):
                            f()
                        ks = slice(kt * P, (kt + 1) * P)
                        # scores in four 1-bank [128,512] PSUM tiles so each
                        # gets ONE exp engine and frees independently (the
                        # kt+1 scores WAR only the same slot of kt).
                        scs = [ps_sc.tile([P, 512], f32, tag="sc",
                                          name=f"sc{i}")
                               for i in range(4)]
                        for qh in range(2):
                            q0 = qb * 1024 + qh * 512
                            qs = slice(q0, q0 + 512)
                            nc.tensor.matmul(scs[qh][:], qkT[1][g][sl0, ks],
                                             qkT[0][g][sl0, qs],
                                             start=True, stop=True)
                            nc.tensor.matmul(scs[2 + qh][:],
                                             qkT[1][g][sl1, ks],
                                             qkT[0][g][sl1, qs],
                                             start=True, stop=True)
                        # per-tile engine alternates over (kt, head, qh) so
                        # both engines stay ~50% and every (head, qh) region
                        # mixes both exp flavors across kt.
                        et0 = exp_pool.tile([P, 1024], bf16, tag="exp",
                                            name="et0")
                        et1k = exp1_pool.tile([P, 1024], bf16, tag="exp1",
                                              name="et1")
                        # emission order scs0, scs2, scs1, scs3 with engine
                        # A={scs0,scs3}, B={scs2,scs1}: each engine's FIRST
                        # op frees the first-issued score pair of the next
                        # kt early, so all four kt+1 scores issue wait-free
                        # and pair on the PE array.
                        dve0 = kt % 2 == 1
                        for h, qh, use_dve in ((0, 0, dve0), (1, 0, not dve0),
                                               (0, 1, not dve0), (1, 1, dve0)):
                            ett = et0 if h == 0 else et1k
                            emit_exp(ett[:, qh * 512:(qh + 1) * 512],
                                     scs[2 * h + qh], kt, use_dve)
                        et0s.append(et0)
                        et1.append(et1k)
                        if kt == 4:
                            pvt0 = [ps_pv.tile([VW, 512], f32, tag="pv",
                                               name="pv0") for _ in range(2)]
                        if kt >= 4:
                            ktl = kt - 4   # lagged PV for head0
                            vb0 = h0 * VW
                            for qc in range(2):
                                nc.tensor.matmul(
                                    pvt0[qc][:],
                                    vp[ktl][:, vb0:vb0 + VW],
                                    et0s[ktl][:, qc * 512:(qc + 1) * 512],
                                    start=(ktl == 0), stop=False)
                        if inline_h1:
                            vb1 = h1 * VW
                            for qc in range(2):
                                nc.tensor.matmul(
                                    pvt1_inl[qc][:],
                                    vp[kt][:, vb1:vb1 + VW],
                                    et1[kt][:, qc * 512:(qc + 1) * 512],
                                    start=(kt == 0), stop=(kt == NT - 1))
                    for kt in range(NT - 4, NT):
                        vb0 = h0 * VW
                        for qc in range(2):
                            nc.tensor.matmul(pvt0[qc][:],
                                             vp[kt][:, vb0:vb0 + VW],
                                             et0s[kt][:, qc * 512:(qc + 1) * 512],
                                             start=False, stop=(kt == NT - 1))
                    for qc in range(2):
                        emit_out(h0, qb, qc, pvt0[qc])
                    if inline_h1:
                        for qc in range(2):
                            emit_out(h1, qb, qc, pvt1_inl[qc],
                                     on_dve=(qc == 1))
                    else:
                        pending = (h1, qb, et1)

    nc.compile()
    return nc


def _get_nc():
    if "nc" not in _CACHE:
        _CACHE["nc"] = _build()
    return _CACHE["nc"]


def kernel(hidden_states, attention_mask, Wq, bq, Aq, Bq, Wk, bk, Ak, Bk,
           Wv, bv, Av, Bv):
    from concourse import bass_utils
    import ml_dtypes
    import os

    nc = _get_nc()
    bf = ml_dtypes.bfloat16

    hs = np.asarray(hidden_states, dtype=np.float32)
    am = np.asarray(attention_mask, dtype=np.float32)
    weff = {}
    for n, W, A, B in (("q", Wq, Aq, Bq), ("k", Wk, Ak, Bk), ("v", Wv, Av, Bv)):
        W = np.asarray(W, dtype=np.float32)
        A = np.asarray(A, dtype=np.float32)
        B = np.asarray(B, dtype=np.float32)
        weff[n] = (W + LORA_SCALING * (B @ A)).astype(bf)
    biases = {"q": np.asarray(bq, np.float32), "k": np.asarray(bk, np.float32),
              "v": np.asarray(bv, np.float32)}
    hs16T = [np.ascontiguousarray(hs[b].T.astype(bf)) for b in range(BS)]

    in_maps = []
    for c in range(N_CORES):
        b, hg = divmod(c, 2)
        rows = slice(hg * DP, (hg + 1) * DP)
        mask_cols = np.ascontiguousarray(am[b, 0, 0].reshape(NT, P).T)
        m = {
            "xT16": hs16T[b],
            "mask": mask_cols,
            "bdve": np.ascontiguousarray(DVE_B0 + DVE_BM * mask_cols),
        }
        for n in ("q", "k", "v"):
            m[f"wT{n}"] = np.ascontiguousarray(weff[n][rows].T)
        for n in ("q", "k"):
            m[f"b{n}"] = np.ascontiguousarray(
                biases[n][rows].reshape(NG, P).T)
        in_maps.append(m)

    trace = bool(int(os.environ.get("BASS_KERNEL_TRACE", "0")))
    res = bass_utils.run_bass_kernel_spmd(nc, in_maps,
                                          core_ids=list(range(N_CORES)),
                                          trace=trace)
    _CACHE["last_results"] = res

    output = np.empty((BS, SEQ, HID), dtype=np.float32)
    for c in range(N_CORES):
        b, hg = divmod(c, 2)
        r = res.results[c]["out"]                      # [NH, 65, SEQ]
        o = r[:, :HD, :] / r[:, HD:HD + 1, :]          # [NH, 64, SEQ]
        rows = slice(hg * DP, (hg + 1) * DP)
        output[b, :, rows] = (o.transpose(2, 0, 1).reshape(SEQ, DP)
                              + biases["v"][rows][None, :])
    return output
